# revision 9
# baseline (speedup 1.0000x reference)
"""KimiLinear KDA decode step — Trainium2 Bass kernel (8 NeuronCores).

Problem: B=128 decode batch, HK=HV=32 heads, D=128 head dim, K=4 causal conv.
  1. per-channel causal conv1d update + silu over mixed_qkv (12288 channels)
  2. split q/k/v, l2norm(q)*D^-0.5, l2norm(k)
  3. fused KDA gate g = -exp(A_log)*softplus(forget_gate + dt_bias), b=sigmoid(beta)
  4. gated delta-rule readout:
       S' = S * exp(g);  kv = k @ S';  delta = (v - kv)*b
       o  = q @ (S' + k (x) delta) = q @ S' + (q.k) * delta
     The updated state is never materialized: only two mat-vecs against S plus
     the (q.k) rank-1 correction are needed.

Sharding: data-parallel over batch — 16 batches per core; each core handles all
32 heads of its batch slice with zero cross-core communication (matches the
sharding hint: states shard with batch).

Device data layout ("layout A"): all per-token tensors live in SBUF as
[128 partitions = d (head dim), free = h*16 + b] so that
  - the conv is purely elementwise (channel c = sec*4096 + h*128 + d maps to
    partition d, free (sec,h,b)),
  - q/k/v vectors are matmul-ready on the contraction (d) partition axis,
  - per-(b,h) scalars (norms, q.k) are produced/broadcast with tiny
    ones-matmuls on the otherwise idle TensorE.
Host-side staging only reshapes/transposes/replicates activations (layout
choice at upload time); the model weights (conv_weights / A_log / dt_bias) are
additionally pre-folded (-exp(A_log)) per standard inference weight prep.
All arithmetic on activations happens on device in fp32.

Per core HBM traffic ~37 MB (dominated by the 33.5 MB ssm_state slice) — the
kernel is memory-bound; the 512 per-(b,h) fp32 matmuls (stationary = S[b,h],
moving = [k_gated | q_gated] 2 columns) hide under the DMA stream.
"""

import numpy as np

import concourse.bass as bass
import concourse.bacc as bacc
import concourse.mybir as mybir
from concourse.tile import TileContext
from concourse.bass_utils import run_bass_kernel_spmd

F32 = mybir.dt.float32
AF = mybir.ActivationFunctionType
OP = mybir.AluOpType

NCORES = 8
B, HK, HV, D, CK = 128, 32, 32, 128, 4
SEC = 3                      # q | k | v channel sections of 32 heads each
BC = B // NCORES             # batches per core = 16
NHB = HV * BC                # free columns per section = 512
QKV = (2 * HK + HV) * D      # 12288
GW = 8                       # batches per psum output group (2 groups)

_CACHE = {}


def _build_nc():
    # Bacc (not raw Bass): its compile() splits multi-sem waits into event
    # semaphores — TRN2 instructions carry at most one wait.
    nc = bacc.Bacc("TRN2", target_bir_lowering=False, debug=False)
    xq = nc.declare_dram_parameter("xq", [D, SEC * NHB], F32, isOutput=False)
    cst = nc.declare_dram_parameter("cst", [D, 3 * SEC * NHB], F32, isOutput=False)
    wrep = nc.declare_dram_parameter("wrep", [D, 4 * SEC * NHB], F32, isOutput=False)
    # aux = [forget_gate | dt_bias | -exp(A_log) | beta] side by side
    aux = nc.declare_dram_parameter("aux", [D, 4 * NHB], F32, isOutput=False)
    ssm = nc.declare_dram_parameter("ssm", [BC, HV, D, D], F32, isOutput=False)
    o_out = nc.declare_dram_parameter("o_out", [D, NHB], F32, isOutput=True)

    S3 = SEC * NHB  # 1536

    with TileContext(nc) as tc:
        with (
            tc.tile_pool(name="const", bufs=1) as const,
            tc.tile_pool(name="work", bufs=1) as work,
            tc.tile_pool(name="spool", bufs=4) as spool,
            tc.tile_pool(name="psr", bufs=1, space="PSUM") as psr,
            tc.tile_pool(name="psb", bufs=1, space="PSUM") as psb,
            tc.tile_pool(name="pso", bufs=1, space="PSUM") as pso,
        ):
            # ---- input staging --------------------------------------------
            t_cst = const.tile([D, 3 * S3], F32)
            nc.sync.dma_start(t_cst[:], cst[:])
            t_xq = const.tile([D, S3], F32)
            nc.sync.dma_start(t_xq[:], xq[:])
            t_w = const.tile([D, 4 * S3], F32)
            nc.sync.dma_start(t_w[:], wrep[:])
            t_aux = const.tile([D, 4 * NHB], F32)
            nc.scalar.dma_start(t_aux[:], aux[:])
            t_fg = t_aux[:, 0:NHB]
            t_dtb = t_aux[:, NHB:2 * NHB]
            t_nega = t_aux[:, 2 * NHB:3 * NHB]
            t_beta = t_aux[:, 3 * NHB:4 * NHB]

            ones_c = const.tile([D, 1], F32)
            nc.vector.memset(ones_c[:], 1.0)
            ones_r = const.tile([1, D], F32)
            nc.vector.memset(ones_r[:], 1.0)
            ones_rs = const.tile([1, D], F32)
            nc.vector.memset(ones_rs[:], float(D) ** -0.5)

            # ---- causal conv1d single-step + silu -------------------------
            acc = work.tile([D, S3], F32)
            tmp = work.tile([D, S3], F32)
            nc.vector.tensor_tensor(acc[:], t_cst[:, 0:S3], t_w[:, 0:S3], OP.mult)
            for j in (1, 2):
                nc.vector.tensor_tensor(
                    tmp[:], t_cst[:, j * S3:(j + 1) * S3],
                    t_w[:, j * S3:(j + 1) * S3], OP.mult)
                nc.vector.tensor_tensor(acc[:], acc[:], tmp[:], OP.add)
            nc.vector.tensor_tensor(tmp[:], t_xq[:], t_w[:, 3 * S3:4 * S3], OP.mult)
            nc.vector.tensor_tensor(acc[:], acc[:], tmp[:], OP.add)
            x = work.tile([D, S3], F32)
            nc.scalar.activation(x[:], acc[:], AF.Silu)
            q = x[:, 0:NHB]
            k = x[:, NHB:2 * NHB]
            v = x[:, 2 * NHB:3 * NHB]

            # ---- l2 norms (partition reduce via ones-matmul) --------------
            sq = work.tile([D, 2 * NHB], F32)
            nc.vector.tensor_tensor(sq[:, 0:NHB], q, q, OP.mult)
            nc.vector.tensor_tensor(sq[:, NHB:2 * NHB], k, k, OP.mult)
            nrow = psr.tile([1, 2 * NHB], F32)
            nc.tensor.matmul(nrow[:, 0:NHB], ones_c[:], sq[:, 0:NHB],
                             start=True, stop=True)
            nc.tensor.matmul(nrow[:, NHB:2 * NHB], ones_c[:], sq[:, NHB:2 * NHB],
                             start=True, stop=True)
            neps = work.tile([1, 2 * NHB], F32)
            nc.vector.tensor_scalar_add(neps[:], nrow[:], 1e-6)
            rrow = work.tile([1, 2 * NHB], F32)
            nc.vector.reciprocal(rrow[:], neps[:])
            srow = work.tile([1, 2 * NHB], F32)
            nc.scalar.activation(srow[:], rrow[:], AF.Sqrt)  # rsqrt = sqrt(1/x)

            # broadcast 1/||q||*D^-0.5 and 1/||k|| along partitions
            rb = psb.tile([D, 2 * NHB], F32)
            nc.tensor.matmul(rb[:, 0:NHB], ones_rs[:], srow[:, 0:NHB],
                             start=True, stop=True)
            nc.tensor.matmul(rb[:, NHB:2 * NHB], ones_r[:], srow[:, NHB:2 * NHB],
                             start=True, stop=True)
            qh = work.tile([D, NHB], F32)
            nc.vector.tensor_tensor(qh[:], q, rb[:, 0:NHB], OP.mult)
            kh = work.tile([D, NHB], F32)
            nc.vector.tensor_tensor(kh[:], k, rb[:, NHB:2 * NHB], OP.mult)

            # ---- KDA gate: eg = exp(-exp(A_log)*softplus(fg+dt_bias)) -----
            # no softplus ACT table on this compiler: use the numerically
            # stable split softplus(x) = relu(x) + ln(1 + exp(-|x|)) so exp/ln
            # share one table with the final exp.
            g1 = work.tile([D, NHB], F32)
            nc.vector.tensor_tensor(g1[:], t_fg[:], t_dtb[:], OP.add)
            ga = work.tile([D, NHB], F32)
            nc.scalar.activation(ga[:], g1[:], AF.Abs)
            nc.scalar.activation(ga[:], ga[:], AF.Exp, scale=-1.0)
            nc.scalar.activation(ga[:], ga[:], AF.Ln, bias=1.0)
            gr = work.tile([D, NHB], F32)
            nc.vector.tensor_scalar_max(gr[:], g1[:], 0.0)
            sp = work.tile([D, NHB], F32)
            nc.vector.tensor_tensor(sp[:], gr[:], ga[:], OP.add)
            nc.vector.tensor_tensor(g1[:], sp[:], t_nega[:], OP.mult)
            eg = work.tile([D, NHB], F32)
            nc.scalar.activation(eg[:], g1[:], AF.Exp)

            # interleaved moving operand: kq[:, 2*bh] = k_gated, [:, 2*bh+1] = q_gated
            kq = work.tile([D, 2 * NHB], F32)
            kq_v = kq.rearrange("p (n two) -> p n two", two=2)
            nc.vector.tensor_tensor(kq_v[:, :, 0], kh[:], eg[:], OP.mult)
            nc.vector.tensor_tensor(kq_v[:, :, 1], qh[:], eg[:], OP.mult)

            # ---- qk = q_hat . k_hat per (b,h), broadcast along partitions -
            nc.vector.tensor_tensor(sq[:, 0:NHB], qh[:], kh[:], OP.mult)
            qkrow = psr.tile([1, NHB], F32)
            nc.tensor.matmul(qkrow[:], ones_c[:], sq[:, 0:NHB],
                             start=True, stop=True)
            qkrs = work.tile([1, NHB], F32)
            nc.vector.tensor_copy(qkrs[:], qkrow[:])
            qkb_ps = psb.tile([D, NHB], F32)
            nc.tensor.matmul(qkb_ps[:], ones_r[:], qkrs[:], start=True, stop=True)
            qkb = work.tile([D, NHB], F32)
            nc.vector.tensor_copy(qkb[:], qkb_ps[:])

            # sigmoid(beta) = 1/(1+exp(-beta)) — reuses the exp table
            bsig = work.tile([D, NHB], F32)
            nc.scalar.activation(bsig[:], t_beta[:], AF.Exp, scale=-1.0)
            nc.vector.tensor_scalar_add(bsig[:], bsig[:], 1.0)
            nc.vector.reciprocal(bsig[:], bsig[:])

            # ---- main loop: stream S, two mat-vecs per (b,h) --------------
            ssm_r = ssm[:].rearrange("b h k v -> b k h v")  # [BC, 128, 32, 128]
            o_t = work.tile([D, NHB], F32)
            T0 = pso.tile([D, 2 * HV * GW], F32)
            T1 = pso.tile([D, 2 * HV * GW], F32)
            Tg = (T0, T1)
            dt_ = work.tile([D, HV, GW], F32)

            v_v = v.rearrange("p (h b) -> p h b", b=BC)
            bs_v = bsig[:].rearrange("p (h b) -> p h b", b=BC)
            qk_v = qkb[:].rearrange("p (h b) -> p h b", b=BC)
            o_v = o_t[:].rearrange("p (h b) -> p h b", b=BC)

            for b in range(BC):
                S = spool.tile([D, HV, D], F32, name="S", tag="S")
                nc.sync.dma_start(S[:], ssm_r[b])
                grp, bl = divmod(b, GW)
                for h in range(HV):
                    col = 2 * (h * GW + bl)
                    bh = 2 * (h * BC + b)
                    nc.tensor.matmul(
                        Tg[grp][:, col:col + 2], S[:, h, :], kq[:, bh:bh + 2],
                        start=True, stop=True)
                if bl == GW - 1:
                    Tv = Tg[grp].rearrange("p (h bl two) -> p h bl two",
                                           bl=GW, two=2)
                    kv = Tv[:, :, :, 0]
                    o1 = Tv[:, :, :, 1]
                    bsel = slice(grp * GW, (grp + 1) * GW)
                    # delta = (v - kv) * sigmoid(beta)
                    nc.vector.scalar_tensor_tensor(
                        dt_[:], kv, -1.0, v_v[:, :, bsel], OP.mult, OP.add)
                    nc.vector.tensor_tensor(dt_[:], dt_[:], bs_v[:, :, bsel],
                                            OP.mult)
                    # o = o1 + qk * delta
                    nc.vector.tensor_tensor(dt_[:], dt_[:], qk_v[:, :, bsel],
                                            OP.mult)
                    nc.vector.tensor_tensor(o_v[:, :, bsel], dt_[:], o1, OP.add)

            nc.sync.dma_start(o_out[:], o_t[:])

    nc.compile()
    return nc


def _prep_act(a):
    """[bc, sec*32*128] activation slice -> [128 d, sec*32*bc] layout A."""
    bc = a.shape[0]
    return np.ascontiguousarray(
        a.reshape(bc, SEC, HV, D).transpose(3, 1, 2, 0).reshape(D, SEC * HV * bc))


def _prep_inputs(mixed_qkv, forget_gate, beta, conv_state, conv_weights,
                 ssm_state, A_log, dt_bias):
    mixed_qkv = np.asarray(mixed_qkv, np.float32)
    forget_gate = np.asarray(forget_gate, np.float32)
    beta = np.asarray(beta, np.float32)
    conv_state = np.asarray(conv_state, np.float32)
    conv_weights = np.asarray(conv_weights, np.float32)
    ssm_state = np.asarray(ssm_state, np.float32)
    A_log = np.asarray(A_log, np.float32)
    dt_bias = np.asarray(dt_bias, np.float32)

    # shared (weight) tensors
    wr = conv_weights.reshape(SEC, HV, D, CK).transpose(3, 2, 0, 1)  # [4,d,sec,h]
    wr = np.broadcast_to(wr[..., None], (CK, D, SEC, HV, BC))
    wrep = np.ascontiguousarray(
        wr.transpose(1, 0, 2, 3, 4).reshape(D, CK * SEC * HV * BC))
    dtb = np.ascontiguousarray(
        np.broadcast_to(dt_bias.reshape(HV, D).T[:, :, None],
                        (D, HV, BC)).reshape(D, NHB))
    nega = np.ascontiguousarray(
        np.broadcast_to((-np.exp(A_log))[None, :, None],
                        (D, HV, BC)).reshape(D, NHB))

    in_maps = []
    for c in range(NCORES):
        cs = slice(c * BC, (c + 1) * BC)
        cst = conv_state[cs]  # [BC, QKV, 3]
        cstp = np.concatenate([_prep_act(cst[:, :, j]) for j in range(CK - 1)],
                              axis=1)
        fgp = np.ascontiguousarray(
            forget_gate[cs].reshape(BC, HV, D).transpose(2, 1, 0).reshape(D, NHB))
        betar = np.ascontiguousarray(
            np.broadcast_to(beta[cs].T[None, :, :], (D, HV, BC)).reshape(D, NHB))
        in_maps.append({
            "xq": _prep_act(mixed_qkv[cs]),
            "cst": np.ascontiguousarray(cstp),
            "wrep": wrep,
            "aux": np.ascontiguousarray(
                np.concatenate([fgp, dtb, nega, betar], axis=1)),
            "ssm": np.ascontiguousarray(ssm_state[cs]),
        })
    return in_maps


def run(trace=False, **inputs):
    if "nc" not in _CACHE:
        _CACHE["nc"] = _build_nc()
    nc = _CACHE["nc"]
    in_maps = _prep_inputs(**inputs)
    res = run_bass_kernel_spmd(nc, in_maps, list(range(NCORES)), trace=trace)
    outs = []
    for c in range(NCORES):
        oc = np.asarray(res.results[c]["o_out"])  # [128, 512]
        outs.append(oc.reshape(D, HV, BC).transpose(2, 1, 0))  # [BC, HV, D]
    return np.concatenate(outs, axis=0), res


def kernel(**inputs) -> np.ndarray:
    out, _ = run(trace=False, **inputs)
    return out
